# revision 1
# baseline (speedup 1.0000x reference)
"""Trainium2 Bass kernel for nn_Decoder (3-layer GNN message-passing decoder).

Sharding: node axis split across 8 cores (2500 nodes/core), weights replicated.
All on-device tensors live in [feature=128 partitions, free] layout; the host
pre-transposes edge/node features (and casts to bf16) so the device never
transposes anything, and transposes the [C, n] output back at the end.

Per-core, per-layer structure (T=500-node tiles, K=32 edge slots/node):
  S        = W1a@h + W1b@nf                       (per-node part of mm1, PE)
  m1[k]    = gelu(W1e@efT[k] + S + b1)            (PE + DVE bcast-add + ACT)
  m2[k]    = gelu(W2@m1[k] + b2)                  (PE + ACT)
  acc      = h + sum_k (W3/30)@m2[k]              (PSUM-accumulated over k, PE;
                                                   h preloaded via identity matmul)
  h        = LN(acc + K*b3/30)                    (stats via ones-matmul over
                                                   partitions; 1/sqrt via exp(-0.5*ln))
  h        = LN(h + do@gelu(di@h)) * mask
"""

import sys
from contextlib import ExitStack

for _p in ("/opt/trn_rl_repo", "/root/.axon_site/_ro/trn_rl_repo"):
    if _p not in sys.path:
        sys.path.append(_p)

import numpy as np
import ml_dtypes

import concourse.bass as bass
import concourse.tile as tile
from concourse import bacc, mybir
from concourse.bass_utils import run_bass_kernel_spmd
from concourse.masks import make_identity

N, K, C, H, L = 20000, 32, 128, 128, 3
NCORES = 8
NPER = N // NCORES          # 2500 nodes per core
T = 500                     # node tile (NPER divisible)
NT = NPER // T              # 5 tiles
KGRP = 2                    # k-slices per psum group (2*512 fp32 = 2 PSUM banks)
SCALE, EPS = 30.0, 1e-5

BF = mybir.dt.bfloat16
F32 = mybir.dt.float32
AF = mybir.ActivationFunctionType
OP = mybir.AluOpType


def _emit(ctx, tc, io, nper, tsz):
    nc = tc.nc
    nt = nper // tsz
    ngrp = K // KGRP

    consts = ctx.enter_context(tc.tile_pool(name="consts", bufs=1))
    efpool = ctx.enter_context(tc.tile_pool(name="ef", bufs=2))
    spool = ctx.enter_context(tc.tile_pool(name="sp", bufs=6))
    mdpool = ctx.enter_context(tc.tile_pool(name="md", bufs=5))
    tmppool = ctx.enter_context(tc.tile_pool(name="tmp", bufs=3))
    stgpool = ctx.enter_context(tc.tile_pool(name="stg", bufs=2))
    psmain = ctx.enter_context(tc.tile_pool(name="psmain", bufs=3, space="PSUM"))
    psacc = ctx.enter_context(tc.tile_pool(name="psacc", bufs=1, space="PSUM"))
    psmisc = ctx.enter_context(tc.tile_pool(name="psmisc", bufs=1, space="PSUM"))

    # ---- persistent SBUF state ----
    nfh = consts.tile([C, nper], BF, tag="nfh")            # node features == h0
    mask_rep = consts.tile([C, nper], BF, tag="maskr")
    h_bufs = [consts.tile([C, nper], BF, tag=f"hbuf{i}", name=f"hbuf{i}")
              for i in range(2)]
    h1_sb = consts.tile([C, nper], BF, tag="h1")
    x2t = consts.tile([C, nper], BF, tag="x2t")            # pre-LN x for stats/apply
    # half-tile m1/m2 staging (gelu1 out, overwritten in place by gelu2 out)
    m12 = [consts.tile([C, K // 2, tsz], BF, tag=f"m12{i}", name=f"m12{i}")
           for i in range(2)]
    mean_sb = consts.tile([C, nper], F32, tag="mean")
    es2_sb = consts.tile([C, nper], F32, tag="es2")
    u_sb = consts.tile([C, nper], F32, tag="u")
    inv_sb = consts.tile([C, nper], BF, tag="inv")

    wts = {}
    for nm in ("w1aT", "w1bT", "w1eT", "w2T", "w3sT", "diwT", "dowT"):
        wt = consts.tile([C, L, H], BF, tag=nm, name=nm)
        for l in range(L):
            nc.sync.dma_start(out=wt[:, l, :], in_=io[nm][l, :, :])
        wts[nm] = wt
    bvec = consts.tile([C, 15], F32, tag="bvec")
    nc.sync.dma_start(out=bvec[:, :], in_=io["bvec"][:, :])
    lnvec = consts.tile([C, 12], F32, tag="lnvec")
    nc.sync.dma_start(out=lnvec[:, :], in_=io["lnvec"][:, :])

    ident = consts.tile([C, C], BF, tag="ident")
    make_identity(nc, ident[:, :])
    ones_t = consts.tile([C, C], BF, tag="ones")
    nc.vector.memset(ones_t[:, :], 1.0)
    eps_sb = consts.tile([C, 1], F32, tag="eps")
    nc.vector.memset(eps_sb[:, :], EPS)

    nc.sync.dma_start(out=nfh[:, :], in_=io["nfT"][:, :])
    _m = io["maskT"]
    _mb = bass.AP(tensor=_m.tensor, offset=_m.offset, ap=[[0, C], _m.ap[1]])
    nc.sync.dma_start(out=mask_rep[:, :], in_=_mb)

    s_tiles = {}

    def emit_S(l, t, h_src):
        sl_ = slice(t * tsz, (t + 1) * tsz)
        s_ps = psmisc.tile([C, 512], F32, tag="psS", name="s_ps")
        nc.tensor.matmul(s_ps[:, 0:tsz], wts["w1aT"][:, l, :], h_src[:, sl_],
                         start=True, stop=False)
        nc.tensor.matmul(s_ps[:, 0:tsz], wts["w1bT"][:, l, :], nfh[:, sl_],
                         start=False, stop=True)
        s_sb = spool.tile([C, tsz], BF, tag="ssb", name="s_sb")
        nc.vector.tensor_copy(out=s_sb[:, :], in_=s_ps[:, 0:tsz])
        s_tiles[(l, t)] = s_sb

    def bcol(base, l):
        return bvec[:, base + l:base + l + 1]

    def lncol(base, l):
        return lnvec[:, base + l:base + l + 1]

    for t in range(nt):
        emit_S(0, t, nfh)

    for l in range(L):
        h_cur = nfh if l == 0 else h_bufs[(l + 1) % 2]
        w1a = wts["w1aT"][:, l, :]
        w1b = wts["w1bT"][:, l, :]
        w1e = wts["w1eT"][:, l, :]
        w2 = wts["w2T"][:, l, :]
        w3s = wts["w3sT"][:, l, :]
        diw = wts["diwT"][:, l, :]
        dow = wts["dowT"][:, l, :]

        # ======== edge phase, per node tile ========
        for t in range(nt):
            n0 = t * tsz
            sl = slice(n0, n0 + tsz)
            ef_sb = efpool.tile([C, K, tsz], BF, tag="ef")
            for q in range(4):
                nc.sync.dma_start(out=ef_sb[:, q * 8:(q + 1) * 8, :],
                                  in_=io["efT"][:, q * 8:(q + 1) * 8, sl])

            s_sb = s_tiles.pop((l, t))
            s_ap = s_sb[:, :]
            s_bcast = bass.AP(tensor=s_ap.tensor, offset=s_ap.offset,
                              ap=[s_ap.ap[0], [0, KGRP], s_ap.ap[1]])

            # phase-batched halves: A=mm1e+addS+gelu1, B=mm2+gelu2 (in place),
            # C=dense k-sum matmul tail.  Order A0 B0 A1 C0 B1 C1 keeps the
            # ACT stream free of head-of-line stalls while the C tails give
            # the PE long dense bursts (HAM warm-up).
            KH = K // 2
            GH = KH // 2  # psum groups per half

            def phase_A(h):
                for gq in range(GH // 2):
                    stg = stgpool.tile([C, 4, 512], F32, tag="stg", name="stg")
                    for g2 in range(2):
                        g = gq * 2 + g2
                        pa = psmain.tile([C, 2, 512], F32, tag="pm", name="pa")
                        for j in range(2):
                            k = h * KH + g * 2 + j
                            nc.tensor.matmul(pa[:, j, 0:tsz], w1e, ef_sb[:, k, :],
                                             start=True, stop=True)
                        nc.vector.tensor_add(stg[:, g2 * 2:(g2 + 1) * 2, 0:tsz],
                                             pa[:, :, 0:tsz], s_bcast)
                    nc.scalar.activation(out=m12[h][:, gq * 4:(gq + 1) * 4, 0:tsz],
                                         in_=stg[:, :, 0:tsz],
                                         func=AF.Gelu, bias=bcol(0, l))

            def phase_B(h):
                for g in range(GH):
                    pb = psmain.tile([C, 2, 512], F32, tag="pm", name="pb")
                    for i in range(2):
                        nc.tensor.matmul(pb[:, i, 0:tsz], w2,
                                         m12[h][:, g * 2 + i, 0:tsz],
                                         start=True, stop=True)
                    nc.scalar.activation(out=m12[h][:, g * 2:(g + 1) * 2, 0:tsz],
                                         in_=pb[:, :, 0:tsz],
                                         func=AF.Gelu, bias=bcol(3, l))

            def phase_C(h, acc):
                for kk in range(KH):
                    nc.tensor.matmul(acc[:, 0:tsz], w3s, m12[h][:, kk, 0:tsz],
                                     start=False, stop=(h == 1 and kk == KH - 1))

            phase_A(0)
            phase_B(0)
            phase_A(1)
            acc_ps = psacc.tile([C, 512], F32, tag="acc", name="acc_ps")
            nc.tensor.matmul(acc_ps[:, 0:tsz], ident[:, :], h_cur[:, sl],
                             start=True, stop=False)
            phase_C(0, acc_ps)
            phase_B(1)
            phase_C(1, acc_ps)
            # x = acc + K*b3/30 -> bf16 ; sq = x*x ; partition sums via ones-matmul
            nc.vector.tensor_scalar(x2t[:, sl], acc_ps[:, 0:tsz],
                                    bcol(6, l), None, OP.add)
            sq = tmppool.tile([C, tsz], BF, tag="sq", name="sq", bufs=6)
            nc.vector.tensor_mul(sq[:, :], x2t[:, sl], x2t[:, sl])
            st1 = psmisc.tile([C, 512], F32, tag="psS", name="st1")
            nc.tensor.matmul(st1[:, 0:tsz], ones_t[:, :], x2t[:, sl],
                             start=True, stop=True)
            nc.vector.tensor_scalar(mean_sb[:, sl], st1[:, 0:tsz],
                                    1.0 / C, None, OP.mult)
            st2 = psmisc.tile([C, 512], F32, tag="psS", name="st2")
            nc.tensor.matmul(st2[:, 0:tsz], ones_t[:, :], sq[:, :],
                             start=True, stop=True)
            nc.vector.tensor_scalar(es2_sb[:, sl], st2[:, 0:tsz],
                                    1.0 / C, None, OP.mult)
            nc.vector.tensor_mul(u_sb[:, sl], mean_sb[:, sl], mean_sb[:, sl])
            nc.vector.tensor_sub(u_sb[:, sl], es2_sb[:, sl], u_sb[:, sl])

        # ======== node phase (per layer), phase-batched ========
        def make_inv():
            # inv = exp(-0.5 * ln(var + eps)); var precomputed into u_sb
            nc.scalar.activation(out=u_sb[:, :], in_=u_sb[:, :], func=AF.Ln,
                                 bias=eps_sb[:, :])
            nc.scalar.activation(out=inv_sb[:, :], in_=u_sb[:, :], func=AF.Exp,
                                 scale=-0.5)

        make_inv()  # LN1
        # pass 1: LN1 apply for all tiles
        for t in range(nt):
            sl = slice(t * tsz, (t + 1) * tsz)
            tmp = tmppool.tile([C, tsz], BF, tag="tmp")
            nc.vector.tensor_sub(tmp[:, :], x2t[:, sl], mean_sb[:, sl])
            nc.vector.tensor_mul(tmp[:, :], tmp[:, :], inv_sb[:, sl])
            nc.vector.tensor_scalar(h1_sb[:, sl], tmp[:, :],
                                    lncol(0, l), lncol(3, l), OP.mult, OP.add)
        # pass 2: di matmul + gelu for all tiles
        mds = []
        for t in range(nt):
            sl = slice(t * tsz, (t + 1) * tsz)
            dpa = psmisc.tile([C, 512], F32, tag="psS", name="dpa")
            nc.tensor.matmul(dpa[:, 0:tsz], diw, h1_sb[:, sl], start=True, stop=True)
            md = mdpool.tile([C, tsz], BF, tag="md", name="md")
            nc.scalar.activation(out=md[:, :], in_=dpa[:, 0:tsz], func=AF.Gelu,
                                 bias=bcol(9, l))
            mds.append(md)
        # pass 3 (sub-phase batched): do-matmuls, then x2/sq, then stats
        for t in range(nt):
            sl = slice(t * tsz, (t + 1) * tsz)
            dpb = psmisc.tile([C, 512], F32, tag="psS", name="dpb")
            nc.tensor.matmul(dpb[:, 0:tsz], ident[:, :], h1_sb[:, sl],
                             start=True, stop=False)
            nc.tensor.matmul(dpb[:, 0:tsz], dow, mds[t][:, :], start=False, stop=True)
            nc.vector.tensor_scalar(x2t[:, sl], dpb[:, 0:tsz],
                                    bcol(12, l), None, OP.add)
        sq2s = []
        for t in range(nt):
            sl = slice(t * tsz, (t + 1) * tsz)
            sq2 = tmppool.tile([C, tsz], BF, tag="sq", name="sq2", bufs=6)
            nc.vector.tensor_mul(sq2[:, :], x2t[:, sl], x2t[:, sl])
            sq2s.append(sq2)
        for t in range(nt):
            sl = slice(t * tsz, (t + 1) * tsz)
            dpc = psmisc.tile([C, 512], F32, tag="psS", name="dpc")
            nc.tensor.matmul(dpc[:, 0:tsz], ones_t[:, :], x2t[:, sl],
                             start=True, stop=True)
            nc.vector.tensor_scalar(mean_sb[:, sl], dpc[:, 0:tsz],
                                    1.0 / C, None, OP.mult)
        for t in range(nt):
            sl = slice(t * tsz, (t + 1) * tsz)
            dpd = psmisc.tile([C, 512], F32, tag="psS", name="dpd")
            nc.tensor.matmul(dpd[:, 0:tsz], ones_t[:, :], sq2s[t][:, :],
                             start=True, stop=True)
            nc.vector.tensor_scalar(es2_sb[:, sl], dpd[:, 0:tsz],
                                    1.0 / C, None, OP.mult)
        for t in range(nt):
            sl = slice(t * tsz, (t + 1) * tsz)
            nc.vector.tensor_mul(u_sb[:, sl], mean_sb[:, sl], mean_sb[:, sl])
            nc.vector.tensor_sub(u_sb[:, sl], es2_sb[:, sl], u_sb[:, sl])

        make_inv()  # LN2
        for t in range(nt):
            sl = slice(t * tsz, (t + 1) * tsz)
            tmp = tmppool.tile([C, tsz], BF, tag="tmp")
            nc.vector.tensor_sub(tmp[:, :], x2t[:, sl], mean_sb[:, sl])
            nc.vector.tensor_mul(tmp[:, :], tmp[:, :], inv_sb[:, sl])
            if l < L - 1:
                q = tmppool.tile([C, tsz], BF, tag="q")
                nc.vector.tensor_scalar(q[:, :], tmp[:, :],
                                        lncol(6, l), lncol(9, l), OP.mult, OP.add)
                nc.vector.tensor_mul(h_bufs[l % 2][:, sl], q[:, :], mask_rep[:, sl])
                emit_S(l + 1, t, h_bufs[l % 2])
            else:
                q = tmppool.tile([C, tsz], BF, tag="q")
                nc.vector.tensor_scalar(q[:, :], tmp[:, :],
                                        lncol(6, l), lncol(9, l), OP.mult, OP.add)
                nc.vector.tensor_mul(mean_sb[:, sl], q[:, :], mask_rep[:, sl])
                nc.sync.dma_start(out=io["out_hT"][:, sl], in_=mean_sb[:, sl])


def build_nc(nper=NPER, tsz=T):
    nc = bacc.Bacc("TRN2", target_bir_lowering=False, debug=False,
                   enable_asserts=False)
    io = {
        "efT": nc.dram_tensor("efT", [C, K, nper], BF, kind="ExternalInput").ap(),
        "nfT": nc.dram_tensor("nfT", [C, nper], BF, kind="ExternalInput").ap(),
        "maskT": nc.dram_tensor("maskT", [1, nper], BF, kind="ExternalInput").ap(),
        "bvec": nc.dram_tensor("bvec", [C, 15], F32, kind="ExternalInput").ap(),
        "lnvec": nc.dram_tensor("lnvec", [C, 12], F32, kind="ExternalInput").ap(),
        "out_hT": nc.dram_tensor("out_hT", [C, nper], F32, kind="ExternalOutput").ap(),
    }
    for nm in ("w1aT", "w1bT", "w1eT", "w2T", "w3sT", "diwT", "dowT"):
        io[nm] = nc.dram_tensor(nm, [L, C, H], BF, kind="ExternalInput").ap()
    with tile.TileContext(nc) as tc:
        with ExitStack() as ctx:
            _emit(ctx, tc, io, nper, tsz)
    nc.compile()
    return nc


def host_prep(inputs, nper=NPER, ncores=NCORES):
    """Shard + lay out inputs for the device. Returns list of per-core in_maps."""
    bf = ml_dtypes.bfloat16
    nf = np.asarray(inputs["node_features"], np.float32)
    ef = np.asarray(inputs["edge_features"], np.float32)
    mask = np.asarray(inputs["mask"], np.float32)
    w1 = np.asarray(inputs["w1"], np.float32)
    w2 = np.asarray(inputs["w2"], np.float32)
    w3 = np.asarray(inputs["w3"], np.float32)
    di_w = np.asarray(inputs["di_w"], np.float32)
    do_w = np.asarray(inputs["do_w"], np.float32)

    def tr(w):  # (L, A, B) -> (L, B, A) contiguous bf16
        return np.ascontiguousarray(w.transpose(0, 2, 1)).astype(bf)

    shared = {
        "w1aT": tr(w1[:, :, 0:C]),
        "w1bT": tr(w1[:, :, C:2 * C]),
        "w1eT": tr(w1[:, :, 3 * C:4 * C]),
        "w2T": tr(w2),
        "w3sT": tr(w3 / SCALE),
        "diwT": tr(di_w),
        "dowT": tr(do_w),
    }
    bvec = np.zeros((C, 15), np.float32)
    lnvec = np.zeros((C, 12), np.float32)
    for l in range(L):
        bvec[:, 0 + l] = np.asarray(inputs["b1"][l], np.float32)
        bvec[:, 3 + l] = np.asarray(inputs["b2"][l], np.float32)
        bvec[:, 6 + l] = np.asarray(inputs["b3"][l], np.float32) * K / SCALE
        bvec[:, 9 + l] = np.asarray(inputs["di_b"][l], np.float32)
        bvec[:, 12 + l] = np.asarray(inputs["do_b"][l], np.float32)
        lnvec[:, 0 + l] = np.asarray(inputs["n1_s"][l], np.float32)
        lnvec[:, 3 + l] = np.asarray(inputs["n1_b"][l], np.float32)
        lnvec[:, 6 + l] = np.asarray(inputs["n2_s"][l], np.float32)
        lnvec[:, 9 + l] = np.asarray(inputs["n2_b"][l], np.float32)
    shared["bvec"] = bvec
    shared["lnvec"] = lnvec

    in_maps = []
    for c in range(ncores):
        sl = slice(c * nper, (c + 1) * nper)
        efc = ef[sl].astype(bf)                              # (nper, K, C)
        in_maps.append(dict(
            efT=np.ascontiguousarray(efc.transpose(2, 1, 0)),  # (C, K, nper)
            nfT=np.ascontiguousarray(nf[sl].T).astype(bf),
            maskT=mask[sl].reshape(1, nper).astype(bf),
            **shared,
        ))
    return in_maps


_NC_CACHE = {}


def kernel(**inputs):
    in_maps = host_prep(inputs)
    if "nc" not in _NC_CACHE:
        _NC_CACHE["nc"] = build_nc()
    nc = _NC_CACHE["nc"]
    res = run_bass_kernel_spmd(nc, in_maps, core_ids=list(range(NCORES)))
    out = np.concatenate([np.asarray(res.results[c]["out_hT"]).T
                          for c in range(NCORES)], axis=0)
    return np.ascontiguousarray(out.astype(np.float32))



# revision 3
# speedup vs baseline: 1.2101x; 1.2101x over previous
"""Trainium2 Bass kernel for nn_Decoder (3-layer GNN message-passing decoder).

Sharding: node axis split across 8 cores (2500 nodes/core), weights replicated.
All tensors live in [feature=128 partitions, free] layout, fp16 on device
(fp32 PSUM accumulation); the host pre-transposes and casts.

Structure: one flat software pipeline over (layer, tile) steps, T=500-node
tiles, K=32 edge slots.  Per step:
  E[k]   = W1e@ef[k]                       (PE -> PSUM, groups of 2 slots)
  m1pre  = E + S   (S = W1a@h + W1b@nf)    (DVE PSUM+bcast add -> stg f16)
  m1     = gelu(m1pre + b1)                (ACT, FD=4000 from SBUF)
  m2     = gelu(W2@m1 + b2)                (PE + ACT from PSUM, FD=1000)
  acc    = sum_k (W3/30)@m2[k]             (PE chained accumulation)
  x1     = acc + K*b3/30 + h               (DVE scalar_tensor_tensor)
  LN1: stats via (1/C)-ones matmul, var=E[(x-mean)^2], inv-std via 7-op
       fp16 Newton/Householder on DVE (no ACT table churn; ACT stays in
       the gelu set the whole kernel)
  d      = do@gelu(di'@y1 + di_b') ; x2 = diag(n1_s)@y1 + d + (n1_b+do_b)
  LN2 -> h_next * mask  (last layer: fp32 out)
Node-phase work of step i is emitted woven into edge phases of step i+1 so
ACT/PE/DVE FIFOs never head-of-line block across the layer boundary.
"""

import sys
from contextlib import ExitStack

for _p in ("/opt/trn_rl_repo", "/root/.axon_site/_ro/trn_rl_repo"):
    if _p not in sys.path:
        sys.path.append(_p)

import numpy as np

import concourse.bass as bass
import concourse.tile as tile
from concourse import bacc, mybir
from concourse.bass_utils import run_bass_kernel_spmd

N, K, C, H, L = 20000, 32, 128, 128, 3
NCORES = 8
NPER = N // NCORES          # 2500 nodes per core
T = 500                     # node tile
NT = NPER // T              # 5 tiles
SCALE = 30.0

F16 = mybir.dt.float16
F32 = mybir.dt.float32
AF = mybir.ActivationFunctionType
OP = mybir.AluOpType

# fp16 rsqrt: linear seed + 3rd-order Householder (valid for var in [0.45,1.95])
RSA, RSC = -0.46, 1.53


def _bcast(ap, n):
    """free-dim broadcast of a [C, T] tile to [C, n, T]."""
    return bass.AP(tensor=ap.tensor, offset=ap.offset,
                   ap=[ap.ap[0], [0, n], ap.ap[1]])


def _emit(ctx, tc, io, nper, tsz):
    nc = tc.nc
    nt = nper // tsz

    consts = ctx.enter_context(tc.tile_pool(name="consts", bufs=1))
    efpool = ctx.enter_context(tc.tile_pool(name="ef", bufs=3))
    m12pool = ctx.enter_context(tc.tile_pool(name="m12", bufs=3))
    stgpool = ctx.enter_context(tc.tile_pool(name="stg", bufs=2))
    spool = ctx.enter_context(tc.tile_pool(name="sp", bufs=7))
    x2pool = ctx.enter_context(tc.tile_pool(name="x2p", bufs=3))
    xmpool = ctx.enter_context(tc.tile_pool(name="xmp", bufs=3))
    mupool = ctx.enter_context(tc.tile_pool(name="mup", bufs=4))
    nwpool = ctx.enter_context(tc.tile_pool(name="nwp", bufs=6))
    invpool = ctx.enter_context(tc.tile_pool(name="invp", bufs=3))
    y1pool = ctx.enter_context(tc.tile_pool(name="y1p", bufs=2))
    mdpool = ctx.enter_context(tc.tile_pool(name="mdp", bufs=2))
    tmppool = ctx.enter_context(tc.tile_pool(name="tmp", bufs=4))
    outfpool = ctx.enter_context(tc.tile_pool(name="outf", bufs=2))
    pm = ctx.enter_context(tc.tile_pool(name="pm", bufs=3, space="PSUM"))
    psacc = ctx.enter_context(tc.tile_pool(name="psacc", bufs=1, space="PSUM"))
    psmisc = ctx.enter_context(tc.tile_pool(name="psmisc", bufs=1, space="PSUM"))

    # ---- persistent SBUF state ----
    nfh = consts.tile([C, nper], F16, tag="nfh")
    mask_rep = consts.tile([C, nper], F16, tag="maskr")
    h_bufs = [consts.tile([C, nper], F16, tag=f"hbuf{i}", name=f"hbuf{i}")
              for i in range(2)]

    wts = {}
    for nm in ("w1aT", "w1bT", "w1eT", "w2T", "w3sT", "dipT", "dowT"):
        wt = consts.tile([C, L, H], F16, tag=nm, name=nm)
        for l in range(L):
            nc.sync.dma_start(out=wt[:, l, :], in_=io[nm][l, :, :])
        wts[nm] = wt
    diagT = consts.tile([C, L, C], F16, tag="diagT")
    for l in range(L):
        nc.sync.dma_start(out=diagT[:, l, :], in_=io["diagT"][l, :, :])
    bvec = consts.tile([C, 21], F32, tag="bvec")
    nc.sync.dma_start(out=bvec[:, :], in_=io["bvec"][:, :])

    ones_t = consts.tile([C, C], F16, tag="ones")
    nc.vector.memset(ones_t[:, :], 1.0 / C)

    nc.sync.dma_start(out=nfh[:, :], in_=io["nfT"][:, :])
    _m = io["maskT"]
    _mb = bass.AP(tensor=_m.tensor, offset=_m.offset, ap=[[0, C], _m.ap[1]])
    nc.sync.dma_start(out=mask_rep[:, :], in_=_mb)

    def bcol(base, l):
        return bvec[:, base + l:base + l + 1]

    s_tiles = {}
    ef_tiles = {}

    def h_of(l):
        return nfh if l == 0 else h_bufs[(l + 1) % 2]

    def emit_S(l, t):
        sl = slice(t * tsz, (t + 1) * tsz)
        s_ps = psmisc.tile([C, 512], F32, tag="ms", name="s_ps")
        nc.tensor.matmul(s_ps[:, 0:tsz], wts["w1aT"][:, l, :], h_of(l)[:, sl],
                         start=True, stop=False)
        nc.tensor.matmul(s_ps[:, 0:tsz], wts["w1bT"][:, l, :], nfh[:, sl],
                         start=False, stop=True)
        s_sb = spool.tile([C, tsz], F16, tag="ssb", name="s_sb")
        nc.vector.tensor_copy(out=s_sb[:, :], in_=s_ps[:, 0:tsz])
        s_tiles[(l, t)] = s_sb

    def dma_ef(l, t):
        sl = slice(t * tsz, (t + 1) * tsz)
        halves = []
        for h in range(2):
            ef_sb = efpool.tile([C, 16, tsz], F16, tag="ef", name=f"ef{h}")
            for q in range(2):
                k0 = h * 16 + q * 8
                nc.sync.dma_start(out=ef_sb[:, q * 8:(q + 1) * 8, :],
                                  in_=io["efT"][:, k0:k0 + 8, sl])
            halves.append(ef_sb)
        ef_tiles[(l, t)] = halves

    def edge_A(l, t, h, m12buf):
        """mm1e for one 16-slot half + S-adds into stg + gelu1 -> m12buf."""
        ef_sb = ef_tiles[(l, t)][h]
        s_sb = s_tiles[(l, t)]
        sb2 = _bcast(s_sb[:, :], 2)
        w1e = wts["w1eT"][:, l, :]
        for q in range(2):
            stgb = stgpool.tile([C, 8, tsz], F16, tag="stg", name="stg")
            for g in range(4):
                pa = pm.tile([C, 2, 512], F32, tag="pm", name="pa")
                for j in range(2):
                    k = q * 8 + g * 2 + j
                    nc.tensor.matmul(pa[:, j, 0:tsz], w1e, ef_sb[:, k, :],
                                     start=True, stop=True)
                nc.vector.tensor_add(stgb[:, g * 2:(g + 1) * 2, :],
                                     pa[:, :, 0:tsz], sb2)
            nc.scalar.activation(out=m12buf[:, q * 8:(q + 1) * 8, :],
                                 in_=stgb[:, :, :], func=AF.Gelu,
                                 bias=bcol(0, l))

    def edge_B(l, t, m12buf):
        """mm2 + gelu2 (in place) for a 16-slot half."""
        w2 = wts["w2T"][:, l, :]
        for g in range(8):
            pb = pm.tile([C, 2, 512], F32, tag="pm", name="pb")
            for j in range(2):
                nc.tensor.matmul(pb[:, j, 0:tsz], w2,
                                 m12buf[:, g * 2 + j, :], start=True, stop=True)
            nc.scalar.activation(out=m12buf[:, g * 2:(g + 1) * 2, :],
                                 in_=pb[:, :, 0:tsz], func=AF.Gelu,
                                 bias=bcol(3, l))

    def edge_C(l, t, m12buf, acc, first, last):
        w3s = wts["w3sT"][:, l, :]
        for kk in range(16):
            nc.tensor.matmul(acc[:, 0:tsz], w3s, m12buf[:, kk, :],
                             start=(first and kk == 0),
                             stop=(last and kk == 15))

    def newton_inv(u_t):
        """inv = rsqrt(u): linear seed + 3rd-order Householder, fp16 DVE."""
        y = nwpool.tile([C, tsz], F16, tag="nw", name="y")
        nc.vector.tensor_scalar(y[:, :], u_t[:, :], RSA, RSC, OP.mult, OP.add)
        y2 = nwpool.tile([C, tsz], F16, tag="nw", name="y2")
        nc.vector.tensor_mul(y2[:, :], y[:, :], y[:, :])
        w = nwpool.tile([C, tsz], F16, tag="nw", name="w")
        nc.vector.tensor_mul(w[:, :], y2[:, :], u_t[:, :])
        e = nwpool.tile([C, tsz], F16, tag="nw", name="e")
        nc.vector.tensor_scalar(e[:, :], w[:, :], -1.0, 1.0, OP.mult, OP.add)
        p = nwpool.tile([C, tsz], F16, tag="nw", name="p")
        nc.vector.tensor_scalar(p[:, :], e[:, :], 0.375, 0.5, OP.mult, OP.add)
        tt = nwpool.tile([C, tsz], F16, tag="nw", name="tt")
        nc.vector.tensor_mul(tt[:, :], e[:, :], p[:, :])
        inv = invpool.tile([C, tsz], F16, tag="inv", name="inv")
        nc.vector.scalar_tensor_tensor(
            out=inv[:, :], in0=tt[:, :], scalar=1.0, in1=y[:, :],
            op0=OP.add, op1=OP.mult)
        return inv

    def ln_stats(x_t):
        """mean + xm + var=E[xm^2] + inv-std for one [C,tsz] tile."""
        ms = psmisc.tile([C, 512], F32, tag="ms", name="st1")
        nc.tensor.matmul(ms[:, 0:tsz], ones_t[:, :], x_t[:, :],
                         start=True, stop=True)
        mean = mupool.tile([C, tsz], F16, tag="mu", name="mean")
        nc.vector.tensor_copy(out=mean[:, :], in_=ms[:, 0:tsz])
        xm = xmpool.tile([C, tsz], F16, tag="xm", name="xm")
        nc.vector.tensor_sub(xm[:, :], x_t[:, :], mean[:, :])
        sq = tmppool.tile([C, tsz], F16, tag="tmp", name="sq")
        nc.vector.tensor_mul(sq[:, :], xm[:, :], xm[:, :])
        ms2 = psmisc.tile([C, 512], F32, tag="ms", name="st2")
        nc.tensor.matmul(ms2[:, 0:tsz], ones_t[:, :], sq[:, :],
                         start=True, stop=True)
        u = mupool.tile([C, tsz], F16, tag="mu", name="u")
        nc.vector.tensor_copy(out=u[:, :], in_=ms2[:, 0:tsz])
        return xm, newton_inv(u)

    # deferred node-phase state per step: dict with acc, x1, xm, inv, y1, md...
    class Node:
        pass

    def node_x1(st):
        """drain acc -> x1 (STT: acc + b3K + h)."""
        l, t = st.l, st.t
        sl = slice(t * tsz, (t + 1) * tsz)
        x1 = x2pool.tile([C, tsz], F16, tag="x2", name="x1")
        nc.vector.scalar_tensor_tensor(
            out=x1[:, :], in0=st.acc[:, 0:tsz], scalar=bcol(6, l),
            in1=h_of(l)[:, sl], op0=OP.add, op1=OP.add)
        st.x1 = x1

    def node_mid(st):
        """LN1 stats+inv+apply, dpa matmul (dense gelu input)."""
        l = st.l
        xm, inv = ln_stats(st.x1)
        y1 = y1pool.tile([C, tsz], F16, tag="y1", name="y1")
        nc.vector.tensor_mul(y1[:, :], xm[:, :], inv[:, :])
        st.y1 = y1
        dpa = psmisc.tile([C, 512], F32, tag="ms", name="dpa")
        nc.tensor.matmul(dpa[:, 0:tsz], wts["dipT"][:, l, :], y1[:, :],
                         start=True, stop=True)
        st.dpa = dpa

    def node_md(st):
        md = mdpool.tile([C, tsz], F16, tag="md", name="md")
        nc.scalar.activation(out=md[:, :], in_=st.dpa[:, 0:tsz], func=AF.Gelu,
                             bias=bcol(9, st.l))
        st.md = md

    def node_dpb(st):
        l = st.l
        dpb = psmisc.tile([C, 512], F32, tag="ms", name="dpb")
        nc.tensor.matmul(dpb[:, 0:tsz], diagT[:, l, :], st.y1[:, :],
                         start=True, stop=False)
        nc.tensor.matmul(dpb[:, 0:tsz], wts["dowT"][:, l, :], st.md[:, :],
                         start=False, stop=True)
        st.dpb = dpb

    def node_fin(st):
        """x2, LN2, h_next (or output), S for next layer."""
        l, t = st.l, st.t
        sl = slice(t * tsz, (t + 1) * tsz)
        x2 = x2pool.tile([C, tsz], F16, tag="x2", name="x2")
        nc.vector.tensor_scalar(x2[:, :], st.dpb[:, 0:tsz], bcol(12, l), None,
                                OP.add)
        xm2, inv2 = ln_stats(x2)
        y2 = tmppool.tile([C, tsz], F16, tag="tmp", name="y2")
        nc.vector.tensor_mul(y2[:, :], xm2[:, :], inv2[:, :])
        t2 = tmppool.tile([C, tsz], F16, tag="tmp", name="t2")
        nc.vector.tensor_scalar(t2[:, :], y2[:, :], bcol(15, l), bcol(18, l),
                                OP.mult, OP.add)
        if l < L - 1:
            nc.vector.tensor_mul(h_bufs[l % 2][:, sl], t2[:, :],
                                 mask_rep[:, sl])
            emit_S(l + 1, t)
        else:
            outf = outfpool.tile([C, tsz], F32, tag="outf", name="outf")
            nc.vector.tensor_mul(outf[:, :], t2[:, :], mask_rep[:, sl])
            nc.sync.dma_start(out=io["out_hT"][:, sl], in_=outf[:, :])

    # ---------------- driver ----------------
    steps = [(l, t) for l in range(L) for t in range(NT)]

    for t in range(nt):
        emit_S(0, t)
    dma_ef(*steps[0])
    dma_ef(*steps[1])

    prev = None
    for i, (l, t) in enumerate(steps):
        if i + 2 < len(steps):
            dma_ef(*steps[i + 2])
        m12_h0 = m12pool.tile([C, 16, tsz], F16, tag="m12", name="m12h0")
        edge_A(l, t, 0, m12_h0)
        m12_h1 = m12pool.tile([C, 16, tsz], F16, tag="m12", name="m12h1")
        edge_A(l, t, 1, m12_h1)
        if prev is not None:
            node_x1(prev)
        edge_B(l, t, m12_h0)
        if prev is not None:
            node_mid(prev)
            node_md(prev)
        acc = psacc.tile([C, 512], F32, tag="acc", name="acc")
        edge_C(l, t, m12_h0, acc, first=True, last=False)
        edge_B(l, t, m12_h1)
        if prev is not None:
            node_dpb(prev)
        edge_C(l, t, m12_h1, acc, first=False, last=True)
        if prev is not None:
            node_fin(prev)
        ef_tiles.pop((l, t))
        st = Node()
        st.l, st.t, st.acc = l, t, acc
        prev = st
    # drain the last step's node phase
    node_x1(prev)
    node_mid(prev)
    node_md(prev)
    node_dpb(prev)
    node_fin(prev)


def build_nc(nper=NPER, tsz=T):
    nc = bacc.Bacc("TRN2", target_bir_lowering=False, debug=False,
                   enable_asserts=False)
    io = {
        "efT": nc.dram_tensor("efT", [C, K, nper], F16, kind="ExternalInput").ap(),
        "nfT": nc.dram_tensor("nfT", [C, nper], F16, kind="ExternalInput").ap(),
        "maskT": nc.dram_tensor("maskT", [1, nper], F16, kind="ExternalInput").ap(),
        "bvec": nc.dram_tensor("bvec", [C, 21], F32, kind="ExternalInput").ap(),
        "diagT": nc.dram_tensor("diagT", [L, C, C], F16, kind="ExternalInput").ap(),
        "out_hT": nc.dram_tensor("out_hT", [C, nper], F32, kind="ExternalOutput").ap(),
    }
    for nm in ("w1aT", "w1bT", "w1eT", "w2T", "w3sT", "dipT", "dowT"):
        io[nm] = nc.dram_tensor(nm, [L, C, H], F16, kind="ExternalInput").ap()
    with tile.TileContext(nc) as tc:
        with ExitStack() as ctx:
            _emit(ctx, tc, io, nper, tsz)
    nc.compile()
    return nc


def host_prep(inputs, nper=NPER, ncores=NCORES):
    """Shard + lay out inputs for the device. Returns list of per-core in_maps."""
    f16 = np.float16
    nf = np.asarray(inputs["node_features"], np.float32)
    ef = np.asarray(inputs["edge_features"], np.float32)
    mask = np.asarray(inputs["mask"], np.float32)
    w1 = np.asarray(inputs["w1"], np.float32)
    w2 = np.asarray(inputs["w2"], np.float32)
    w3 = np.asarray(inputs["w3"], np.float32)
    di_w = np.asarray(inputs["di_w"], np.float32)
    di_b = np.asarray(inputs["di_b"], np.float32)
    do_w = np.asarray(inputs["do_w"], np.float32)
    n1_s = np.asarray(inputs["n1_s"], np.float32)
    n1_b = np.asarray(inputs["n1_b"], np.float32)

    def tr(w):  # (L, A, B) -> (L, B, A) contiguous f16
        return np.ascontiguousarray(w.transpose(0, 2, 1)).astype(f16)

    dip = di_w * n1_s[:, None, :]                       # fold LN1 scale
    dibp = di_b + np.einsum('lhc,lc->lh', di_w, n1_b)   # fold LN1 bias
    diag = np.zeros((L, C, C), np.float32)
    for l in range(L):
        np.fill_diagonal(diag[l], n1_s[l])

    shared = {
        "w1aT": tr(w1[:, :, 0:C]),
        "w1bT": tr(w1[:, :, C:2 * C]),
        "w1eT": tr(w1[:, :, 3 * C:4 * C]),
        "w2T": tr(w2),
        "w3sT": tr(w3 / SCALE),
        "dipT": tr(dip),
        "dowT": tr(do_w),
        "diagT": diag.astype(f16),   # [L, C, C]; lhsT of diag matmul (symmetric)
    }
    bvec = np.zeros((C, 21), np.float32)
    for l in range(L):
        bvec[:, 0 + l] = np.asarray(inputs["b1"][l], np.float32)
        bvec[:, 3 + l] = np.asarray(inputs["b2"][l], np.float32)
        bvec[:, 6 + l] = np.asarray(inputs["b3"][l], np.float32) * K / SCALE
        bvec[:, 9 + l] = dibp[l]
        bvec[:, 12 + l] = n1_b[l] + np.asarray(inputs["do_b"][l], np.float32)
        bvec[:, 15 + l] = np.asarray(inputs["n2_s"][l], np.float32)
        bvec[:, 18 + l] = np.asarray(inputs["n2_b"][l], np.float32)
    shared["bvec"] = bvec

    in_maps = []
    for c in range(ncores):
        sl = slice(c * nper, (c + 1) * nper)
        efc = ef[sl].astype(f16)                             # (nper, K, C)
        in_maps.append(dict(
            efT=np.ascontiguousarray(efc.transpose(2, 1, 0)),  # (C, K, nper)
            nfT=np.ascontiguousarray(nf[sl].T).astype(f16),
            maskT=mask[sl].reshape(1, nper).astype(f16),
            **shared,
        ))
    return in_maps


_NC_CACHE = {}


def kernel(**inputs):
    in_maps = host_prep(inputs)
    if "nc" not in _NC_CACHE:
        _NC_CACHE["nc"] = build_nc()
    nc = _NC_CACHE["nc"]
    res = run_bass_kernel_spmd(nc, in_maps, core_ids=list(range(NCORES)))
    out = np.concatenate([np.asarray(res.results[c]["out_hT"]).T
                          for c in range(NCORES)], axis=0)
    return np.ascontiguousarray(out.astype(np.float32))
